# revision 10
# baseline (speedup 1.0000x reference)
"""TRN2 Bass kernel for nn_Attention (cross-attention, Tq=2, Tk=5, B=16384, D=512).

Math reformulation (exact):
    q~ = h @ W_A,        W_A  = Wq @ Wk^T          (host-precomputed, tiny)
    logits[b,i,j] = q~[b,i,:] . e[b,j,:]           (DVE dots, fp32 accum)
    ex = exp(logits - max)                          (Act)
    ctxu[b,i,:] = sum_j ex[b,i,j] * e[b,j,:]       (PE: diag(ex) matmuls, PSUM accum)
    ctx = ctxu / sum_j ex                           (folded into Act PSUM->SBUF copy)
    out = h @ Wd1 + ctx @ W_vd,  W_vd = Wv @ Wd2   (host-precomputed, tiny)

Per-batch weighted sums run on the PE via diagonal stationary matrices:
    matmul(psum, lhsT=diag(ex_ij), rhs=e_j)  accumulates ex_ij[b]*e[b,j,:] per lane.
diag(ex_ij) is a single-scalar 4x-mode tensor_scalar op on a fp16 identity.
Softmax normalization rides the Act-engine copy (per-partition scale = 1/sum).

Sharding: pure data parallel over batch, 2048 per core x 8 cores.
Host marshals e to batch-major [B, Tk, D] fp16 and h to block-transposed
lhsT layout [NT, P(d), DC, Tq, P(b)] fp16. Output fp16, upcast on host.
Main loop is a 3-stage software pipeline (A: loads+q~ | B: dots+max+exp |
C: recip+diag+ctx+transpose+out) so the DVE never stalls on Act's EXP.
"""

import numpy as np

import concourse.bass as bass
import concourse.mybir as mybir
import concourse.tile as tile
from concourse import bacc
from concourse.bass_utils import run_bass_kernel_spmd
from concourse.masks import make_identity

F32 = mybir.dt.float32
F16 = mybir.dt.float16
MUL = mybir.AluOpType.mult
ADD = mybir.AluOpType.add
BYP = mybir.AluOpType.bypass

TQ, TK, B, D = 2, 5, 16384, 512
NCORES = 8
BL = B // NCORES          # 2048 batch per core
P = 128                   # partition tile
NT = BL // P              # 16 batch tiles per core
DC = D // P               # 4 contraction chunks

_CACHED = {}


def build():
    nc = bacc.Bacc("TRN2", target_bir_lowering=False, debug=False)

    e_d = nc.dram_tensor("enc", [BL, TK, D], F16, kind="ExternalInput")
    ht_d = nc.dram_tensor("hT", [NT, P, DC, TQ, P], F16, kind="ExternalInput")
    wqk_d = nc.dram_tensor("Wqk", [P, DC, D], F16, kind="ExternalInput")
    wd1_d = nc.dram_tensor("Wd1", [P, DC, D], F16, kind="ExternalInput")
    wvd_d = nc.dram_tensor("Wvd", [P, DC, D], F16, kind="ExternalInput")
    o_d = nc.dram_tensor("out", [BL, TQ, D], F16, kind="ExternalOutput")

    e_r = e_d.ap()
    o_r = o_d.ap()

    with tile.TileContext(nc) as tc:
        with (
            tc.tile_pool(name="wgt", bufs=1) as wgt,
            tc.tile_pool(name="io", bufs=4) as io,
            tc.tile_pool(name="qp", bufs=4) as qp,
            tc.tile_pool(name="work", bufs=2) as work,
            tc.tile_pool(name="small", bufs=4) as small,
            tc.tile_pool(name="scr", bufs=2) as scr,
            tc.tile_pool(name="obp", bufs=2) as obp,
            tc.tile_pool(name="psq", bufs=1, space="PSUM") as psq,   # [P,TQ,D]f32 2bk
            tc.tile_pool(name="psc", bufs=2, space="PSUM") as psc,   # [P,TQ,D]f32 2bk x2
            tc.tile_pool(name="pso", bufs=2, space="PSUM") as pso,   # [P,D]f32 1bk x2
        ):
            ident = wgt.tile([P, P], F16)
            make_identity(nc, ident)

            wqk = wgt.tile([P, DC, D], F16, tag="wqk")
            wd1 = wgt.tile([P, DC, D], F16, tag="wd1")
            wvd = wgt.tile([P, DC, D], F16, tag="wvd")
            nc.gpsimd.dma_start(out=wqk, in_=wqk_d.ap())
            nc.gpsimd.dma_start(out=wd1, in_=wd1_d.ap())
            nc.gpsimd.dma_start(out=wvd, in_=wvd_d.ap())

            def tt_accum(out, in0, in1, op, accum_out):
                """tensor_tensor with accumulator readout (2x-mode capable).

                Same emission as scalar_tensor_tensor's accum_out path, but on
                the plain TENSOR_TENSOR opcode whose 2x_1p uop exists; the
                [P,1] fp32 accumulator output is exempt from the perf-mode
                stream checks."""
                v = nc.vector
                return v.add_instruction(
                    mybir.InstTensorTensor(
                        name=nc.get_next_instruction_name(),
                        op=op,
                        ins=[v.lower_ap(in0), v.lower_ap(in1)],
                        outs=[v.lower_ap(out), v.lower_ap(accum_out)],
                    ))

            # ================= 3-stage software-pipelined loop =================
            def stage_a(t):
                bsl = slice(t * P, (t + 1) * P)
                en = io.tile([P, TK, D], F16, tag="en", name=f"en{t}")
                nc.sync.dma_start(out=en, in_=e_r[bsl])
                hT = io.tile([P, DC, TQ, P], F16, tag="hT", name=f"hT{t}")
                nc.sync.dma_start(out=hT, in_=ht_d.ap()[t])

                # q~ = h @ W_A   [P, TQ, D]
                pq = psq.tile([P, TQ, D], F32, tag="pq", name=f"pq{t}")
                for i in range(TQ):
                    for c in range(DC):
                        nc.tensor.matmul(
                            pq[:, i, :], hT[:, c, i, :], wqk[:, c, :],
                            start=(c == 0), stop=(c == DC - 1))
                qn = qp.tile([P, TQ, D], F16, tag="qn", name=f"qn{t}")
                nc.scalar.copy(qn, pq)

                return dict(t=t, en=en, hT=hT, qn=qn)

            def stage_b(st):
                t, en, qn = st["t"], st["en"], st["qn"]

                # logits[b,i,j] = q~_i . e_j  (DVE 1x dots, fp32 accumulator)
                lg = small.tile([P, TQ, TK], F32, tag="lg", name=f"lg{t}")
                dump = scr.tile([P, D], F16, tag="dump", name=f"du{t}")
                for i in range(TQ):
                    for j in range(TK):
                        nc.vector.scalar_tensor_tensor(
                            out=dump,
                            in0=qn[:, i, :], scalar=1.0, in1=en[:, j, :],
                            op0=BYP, op1=MUL,
                            accum_out=lg[:, i, j:j + 1])

                nmx = small.tile([P, TQ], F32, tag="nmx", name=f"nm{t}")
                nc.vector.tensor_reduce(
                    out=nmx, in_=lg, axis=mybir.AxisListType.X,
                    op=mybir.AluOpType.max, negate=True)
                pr = small.tile([P, TQ, TK], F32, tag="pr", name=f"pr{t}")
                sm = small.tile([P, TQ], F32, tag="sm", name=f"sm{t}")
                for i in range(TQ):
                    nc.scalar.activation(
                        out=pr[:, i, :], in_=lg[:, i, :],
                        func=mybir.ActivationFunctionType.Exp,
                        bias=nmx[:, i:i + 1],
                        accum_out=sm[:, i:i + 1])
                st.update(pr=pr, sm=sm)
                return st

            def stage_c(st):
                t, en, hT, pr, sm = st["t"], st["en"], st["hT"], st["pr"], st["sm"]
                bsl = slice(t * P, (t + 1) * P)

                rs = small.tile([P, TQ], F32, tag="rs", name=f"rs{t}")
                nc.vector.reciprocal(rs, sm)

                # diag(ex_ij) = ident * ex_ij (i=0 on DVE 4x, i=1 on Act scale)
                dg = work.tile([P, TQ, TK, P], F16, tag="dg", name=f"dg{t}")
                for j in range(TK):
                    nc.vector.tensor_scalar_mul(
                        dg[:, 0, j, :], ident, pr[:, 0, j:j + 1])
                for j in range(TK):
                    nc.scalar.mul(dg[:, 1, j, :], ident, pr[:, 1, j:j + 1])

                # ctxu_i = sum_j diag(ex_ij) @ e_j   (PE, PSUM accumulation)
                pc = psc.tile([P, TQ, D], F32, tag="pc", name=f"pc{t}")
                for i in range(TQ):
                    for j in range(TK):
                        nc.tensor.matmul(
                            pc[:, i, :], dg[:, i, j, :], en[:, j, :],
                            start=(j == 0), stop=(j == TK - 1))
                # normalize during PSUM->SBUF copy: ctx_i = ctxu_i * (1/sum_i)
                cx = work.tile([P, TQ, D], F16, tag="cx", name=f"cx{t}")
                for i in range(TQ):
                    nc.scalar.mul(cx[:, i, :], pc[:, i, :], rs[:, i:i + 1])

                # transpose ctx -> cT [P(d), DC, TQ, P(b)] via DMA xbar
                cT = work.tile([P, DC, TQ, P], F16, tag="cT", name=f"cT{t}")
                for i in range(TQ):
                    for c in range(DC):
                        nc.sync.dma_start_transpose(
                            out=cT[:, c, i, :],
                            in_=cx[:, i, c * P:(c + 1) * P])

                # out_i = h_i @ Wd1 + ctx_i @ Wvd
                ob = obp.tile([P, TQ, D], F16, tag="ob", name=f"ob{t}")
                for i in range(TQ):
                    po = pso.tile([P, D], F32, tag="po", name=f"po{t}_{i}")
                    for c in range(DC):
                        nc.tensor.matmul(po, hT[:, c, i, :], wd1[:, c, :],
                                         start=(c == 0), stop=False)
                    for c in range(DC):
                        nc.tensor.matmul(po, cT[:, c, i, :], wvd[:, c, :],
                                         start=False, stop=(c == DC - 1))
                    nc.scalar.copy(ob[:, i, :], po)
                nc.sync.dma_start(out=o_r[bsl], in_=ob)

            stA, stB = {}, {}
            for tt in range(NT + 2):
                if tt < NT:
                    stA[tt] = stage_a(tt)
                if 1 <= tt < NT + 1:
                    stB[tt - 1] = stage_b(stA.pop(tt - 1))
                if tt >= 2:
                    stage_c(stB.pop(tt - 2))

    nc.compile()
    return nc


def _pack_w(w):
    # [D, D] fp32 -> [P, DC, D] fp16 with contraction dim on partitions
    return np.ascontiguousarray(
        w.reshape(DC, P, D).transpose(1, 0, 2).astype(np.float16))


def kernel(h, enc_out, Wq, Wk, Wv, Wdown, _trace=False):
    h = np.ascontiguousarray(h, dtype=np.float32)
    enc_out = np.ascontiguousarray(enc_out, dtype=np.float32)
    Wq = np.ascontiguousarray(Wq, dtype=np.float32)
    Wk = np.ascontiguousarray(Wk, dtype=np.float32)
    Wv = np.ascontiguousarray(Wv, dtype=np.float32)
    Wdown = np.ascontiguousarray(Wdown, dtype=np.float32)

    if "nc" not in _CACHED:
        _CACHED["nc"] = build()
    nc = _CACHED["nc"]

    w_a = _pack_w(Wq @ Wk.T)
    w_d1 = _pack_w(Wdown[:D])
    w_vd = _pack_w(Wv @ Wdown[D:])

    h16 = h.astype(np.float16)
    e16 = enc_out.astype(np.float16)
    e_bm = np.ascontiguousarray(e16.transpose(1, 0, 2))        # [B, TK, D]
    # block-transposed lhsT tiles: [core][t][p(d%128)][c][i][p(b%128)]
    hT_bm = np.ascontiguousarray(
        h16.reshape(TQ, NCORES, NT, P, DC, P).transpose(1, 2, 5, 4, 0, 3))
    in_maps = []
    for c in range(NCORES):
        sl = slice(c * BL, (c + 1) * BL)
        in_maps.append({
            "enc": e_bm[sl],
            "hT": hT_bm[c],
            "Wqk": w_a, "Wd1": w_d1, "Wvd": w_vd,
        })

    res = run_bass_kernel_spmd(nc, in_maps, list(range(NCORES)), trace=_trace)
    out_bm = np.concatenate([r["out"] for r in res.results], axis=0)  # [B, TQ, D]
    out = np.ascontiguousarray(out_bm.transpose(1, 0, 2))
    if _trace:
        kernel.last_result = res
    return out.astype(np.float32)


# revision 12
# speedup vs baseline: 1.9912x; 1.9912x over previous
"""TRN2 Bass kernel for nn_Attention (cross-attention, Tq=2, Tk=5, B=16384, D=512).

Math reformulation (exact):
    q~ = h @ W_A,        W_A  = Wq @ Wk^T          (host-precomputed, tiny)
    logits[b,i,j] = q~[b,i,:] . e[b,j,:]           (DVE dots, fp32 accum)
    ex = exp(logits - max)                          (Act)
    ctxu[b,i,:] = sum_j ex[b,i,j] * e[b,j,:]       (PE: diag(ex) matmuls, PSUM accum)
    ctx = ctxu / sum_j ex                           (folded into Act PSUM->SBUF copy)
    out = h @ Wd1 + ctx @ W_vd,  W_vd = Wv @ Wd2   (host-precomputed, tiny)

Per-batch weighted sums run on the PE via diagonal stationary matrices:
    matmul(psum, lhsT=diag(ex_ij), rhs=e_j)  accumulates ex_ij[b]*e[b,j,:] per lane.
diag(ex_ij) is a single-scalar 4x-mode tensor_scalar op on a fp16 identity.
Softmax normalization rides the Act-engine copy (per-partition scale = 1/sum).

Sharding: pure data parallel over batch, 2048 per core x 8 cores.
Host marshals e to batch-major [B, Tk, D] fp16 and h to block-transposed
lhsT layout [NT, P(d), DC, Tq, P(b)] fp16. Output fp16, upcast on host.
Main loop is a 3-stage software pipeline (A: loads+q~ | B: dots+max+exp |
C: recip+diag+ctx+transpose+out) so the DVE never stalls on Act's EXP.
"""

import numpy as np

import concourse.bass as bass
import concourse.mybir as mybir
import concourse.tile as tile
from concourse import bacc
from concourse.bass_utils import run_bass_kernel_spmd
from concourse.masks import make_identity

F32 = mybir.dt.float32
F16 = mybir.dt.float16
MUL = mybir.AluOpType.mult
ADD = mybir.AluOpType.add
BYP = mybir.AluOpType.bypass

TQ, TK, B, D = 2, 5, 16384, 512
NCORES = 8
BL = B // NCORES          # 2048 batch per core
P = 128                   # partition tile
NT = BL // P              # 16 batch tiles per core
DC = D // P               # 4 contraction chunks

_CACHED = {}


def build():
    nc = bacc.Bacc("TRN2", target_bir_lowering=False, debug=False)

    e_d = nc.dram_tensor("enc", [BL, TK, D], F16, kind="ExternalInput")
    ht_d = nc.dram_tensor("hT", [NT, P, DC, TQ, P], F16, kind="ExternalInput")
    wqk_d = nc.dram_tensor("Wqk", [P, DC, D], F16, kind="ExternalInput")
    wd1_d = nc.dram_tensor("Wd1", [P, DC, D], F16, kind="ExternalInput")
    wvd_d = nc.dram_tensor("Wvd", [P, DC, D], F16, kind="ExternalInput")
    o_d = nc.dram_tensor("out", [BL, TQ, D], F16, kind="ExternalOutput")

    e_r = e_d.ap()
    o_r = o_d.ap()

    with tile.TileContext(nc) as tc:
        with (
            tc.tile_pool(name="wgt", bufs=1) as wgt,
            tc.tile_pool(name="io", bufs=4) as io,
            tc.tile_pool(name="qp", bufs=4) as qp,
            tc.tile_pool(name="work", bufs=2) as work,
            tc.tile_pool(name="small", bufs=4) as small,
            tc.tile_pool(name="scr", bufs=2) as scr,
            tc.tile_pool(name="obp", bufs=2) as obp,
            tc.tile_pool(name="psq", bufs=1, space="PSUM") as psq,   # [P,TQ,D]f32 2bk
            tc.tile_pool(name="psc", bufs=1, space="PSUM") as psc,   # [P,TQ,D]f32 2bk
            tc.tile_pool(name="pst", bufs=2, space="PSUM") as pst,   # [P,8,P]f16 1bk x2
            tc.tile_pool(name="pso", bufs=2, space="PSUM") as pso,   # [P,D]f32 1bk x2
        ):
            ident = wgt.tile([P, P], F16)
            make_identity(nc, ident)

            wqk = wgt.tile([P, DC, D], F16, tag="wqk")
            wd1 = wgt.tile([P, DC, D], F16, tag="wd1")
            wvd = wgt.tile([P, DC, D], F16, tag="wvd")
            nc.gpsimd.dma_start(out=wqk, in_=wqk_d.ap())
            nc.gpsimd.dma_start(out=wd1, in_=wd1_d.ap())
            nc.gpsimd.dma_start(out=wvd, in_=wvd_d.ap())

            def tt_accum(out, in0, in1, op, accum_out):
                """tensor_tensor with accumulator readout (2x-mode capable).

                Same emission as scalar_tensor_tensor's accum_out path, but on
                the plain TENSOR_TENSOR opcode whose 2x_1p uop exists; the
                [P,1] fp32 accumulator output is exempt from the perf-mode
                stream checks."""
                v = nc.vector
                return v.add_instruction(
                    mybir.InstTensorTensor(
                        name=nc.get_next_instruction_name(),
                        op=op,
                        ins=[v.lower_ap(in0), v.lower_ap(in1)],
                        outs=[v.lower_ap(out), v.lower_ap(accum_out)],
                    ))

            # ================= 3-stage software-pipelined loop =================
            def stage_a(t):
                bsl = slice(t * P, (t + 1) * P)
                en = io.tile([P, TK, D], F16, tag="en", name=f"en{t}")
                nc.sync.dma_start(out=en, in_=e_r[bsl])
                hT = io.tile([P, DC, TQ, P], F16, tag="hT", name=f"hT{t}")
                nc.sync.dma_start(out=hT, in_=ht_d.ap()[t])

                # q~ = h @ W_A   [P, TQ, D]
                pq = psq.tile([P, TQ, D], F32, tag="pq", name=f"pq{t}")
                for i in range(TQ):
                    for c in range(DC):
                        nc.tensor.matmul(
                            pq[:, i, :], hT[:, c, i, :], wqk[:, c, :],
                            start=(c == 0), stop=(c == DC - 1))
                qn = qp.tile([P, TQ, D], F16, tag="qn", name=f"qn{t}")
                nc.scalar.copy(qn, pq)

                return dict(t=t, en=en, hT=hT, qn=qn)

            def stage_b(st):
                t, en, qn = st["t"], st["en"], st["qn"]

                # logits[b,i,j] = q~_i . e_j  (DVE 1x dots, fp32 accumulator)
                lg = small.tile([P, TQ, TK], F32, tag="lg", name=f"lg{t}")
                dump = scr.tile([P, D], F16, tag="dump", name=f"du{t}")
                for i in range(TQ):
                    for j in range(TK):
                        nc.vector.scalar_tensor_tensor(
                            out=dump,
                            in0=qn[:, i, :], scalar=1.0, in1=en[:, j, :],
                            op0=BYP, op1=MUL,
                            accum_out=lg[:, i, j:j + 1])

                nmx = small.tile([P, TQ], F32, tag="nmx", name=f"nm{t}")
                nc.vector.tensor_reduce(
                    out=nmx, in_=lg, axis=mybir.AxisListType.X,
                    op=mybir.AluOpType.max, negate=True)
                pr = small.tile([P, TQ, TK], F32, tag="pr", name=f"pr{t}")
                sm = small.tile([P, TQ], F32, tag="sm", name=f"sm{t}")
                for i in range(TQ):
                    nc.scalar.activation(
                        out=pr[:, i, :], in_=lg[:, i, :],
                        func=mybir.ActivationFunctionType.Exp,
                        bias=nmx[:, i:i + 1],
                        accum_out=sm[:, i:i + 1])
                st.update(pr=pr, sm=sm)
                return st

            def stage_c(st):
                t, en, hT, pr, sm = st["t"], st["en"], st["hT"], st["pr"], st["sm"]
                bsl = slice(t * P, (t + 1) * P)

                rs = small.tile([P, TQ], F32, tag="rs", name=f"rs{t}")
                nc.vector.reciprocal(rs, sm)

                # diag(ex_ij) = ident * ex_ij (i=0 on DVE 4x, i=1 on Act scale)
                dg = work.tile([P, TQ, TK, P], F16, tag="dg", name=f"dg{t}")
                for j in range(TK):
                    nc.vector.tensor_scalar_mul(
                        dg[:, 0, j, :], ident, pr[:, 0, j:j + 1])
                for j in range(TK):
                    nc.scalar.mul(dg[:, 1, j, :], ident, pr[:, 1, j:j + 1])

                # ctxu_i = sum_j diag(ex_ij) @ e_j   (PE, PSUM accumulation)
                pc = psc.tile([P, TQ, D], F32, tag="pc", name=f"pc{t}")
                for i in range(TQ):
                    for j in range(TK):
                        nc.tensor.matmul(
                            pc[:, i, :], dg[:, i, j, :], en[:, j, :],
                            start=(j == 0), stop=(j == TK - 1))
                # normalize during PSUM->SBUF copy: ctx_i = ctxu_i * (1/sum_i)
                cx = work.tile([P, TQ, D], F16, tag="cx", name=f"cx{t}")
                for i in range(TQ):
                    nc.scalar.mul(cx[:, i, :], pc[:, i, :], rs[:, i:i + 1])

                # transpose ctx -> cT [P(d), DC, TQ, P(b)]
                pt = pst.tile([P, TQ * DC, P], F16, tag="pt", name=f"pt{t}")
                for i in range(TQ):
                    for c in range(DC):
                        nc.tensor.transpose(
                            pt[:, i * DC + c, :],
                            cx[:, i, c * P:(c + 1) * P], ident)
                cT = work.tile([P, DC, TQ, P], F16, tag="cT", name=f"cT{t}")
                nc.scalar.copy(cT, pt.rearrange("p (i c) b -> p c i b", i=TQ))

                # out_i = h_i @ Wd1 + ctx_i @ Wvd
                ob = obp.tile([P, TQ, D], F16, tag="ob", name=f"ob{t}")
                for i in range(TQ):
                    po = pso.tile([P, D], F32, tag="po", name=f"po{t}_{i}")
                    for c in range(DC):
                        nc.tensor.matmul(po, hT[:, c, i, :], wd1[:, c, :],
                                         start=(c == 0), stop=False)
                    for c in range(DC):
                        nc.tensor.matmul(po, cT[:, c, i, :], wvd[:, c, :],
                                         start=False, stop=(c == DC - 1))
                    nc.scalar.copy(ob[:, i, :], po)
                nc.sync.dma_start(out=o_r[bsl], in_=ob)

            stA, stB = {}, {}
            for tt in range(NT + 2):
                if tt < NT:
                    stA[tt] = stage_a(tt)
                if 1 <= tt < NT + 1:
                    stB[tt - 1] = stage_b(stA.pop(tt - 1))
                if tt >= 2:
                    stage_c(stB.pop(tt - 2))

    nc.compile()
    return nc


def _pack_w(w):
    # [D, D] fp32 -> [P, DC, D] fp16 with contraction dim on partitions
    return np.ascontiguousarray(
        w.reshape(DC, P, D).transpose(1, 0, 2).astype(np.float16))


def kernel(h, enc_out, Wq, Wk, Wv, Wdown, _trace=False):
    h = np.ascontiguousarray(h, dtype=np.float32)
    enc_out = np.ascontiguousarray(enc_out, dtype=np.float32)
    Wq = np.ascontiguousarray(Wq, dtype=np.float32)
    Wk = np.ascontiguousarray(Wk, dtype=np.float32)
    Wv = np.ascontiguousarray(Wv, dtype=np.float32)
    Wdown = np.ascontiguousarray(Wdown, dtype=np.float32)

    if "nc" not in _CACHED:
        _CACHED["nc"] = build()
    nc = _CACHED["nc"]

    w_a = _pack_w(Wq @ Wk.T)
    w_d1 = _pack_w(Wdown[:D])
    w_vd = _pack_w(Wv @ Wdown[D:])

    h16 = h.astype(np.float16)
    e16 = enc_out.astype(np.float16)
    e_bm = np.ascontiguousarray(e16.transpose(1, 0, 2))        # [B, TK, D]
    # block-transposed lhsT tiles: [core][t][p(d%128)][c][i][p(b%128)]
    hT_bm = np.ascontiguousarray(
        h16.reshape(TQ, NCORES, NT, P, DC, P).transpose(1, 2, 5, 4, 0, 3))
    in_maps = []
    for c in range(NCORES):
        sl = slice(c * BL, (c + 1) * BL)
        in_maps.append({
            "enc": e_bm[sl],
            "hT": hT_bm[c],
            "Wqk": w_a, "Wd1": w_d1, "Wvd": w_vd,
        })

    res = run_bass_kernel_spmd(nc, in_maps, list(range(NCORES)), trace=_trace)
    out_bm = np.concatenate([r["out"] for r in res.results], axis=0)  # [B, TQ, D]
    out = np.ascontiguousarray(out_bm.transpose(1, 0, 2))
    if _trace:
        kernel.last_result = res
    return out.astype(np.float32)
